# revision 3
# baseline (speedup 1.0000x reference)
"""GraphSAGE (3 layers + 3 heads) on 8 Trainium2 NeuronCores.

Strategy (graph/data parallel, per the sharding hint):
- Nodes are sharded across 8 cores by id range (6250 each). Edges are
  assigned to the core that owns their dst node.
- Within a core, nodes are ordered by in-degree (desc) into "slots"
  (padded to 6272 = 49*128). The mean-aggregation becomes a sequence of
  "rounds": round r adds the r-th neighbor's feature row into the
  accumulator of every slot with degree > r. Because slots are sorted by
  degree, round r touches a prefix of slots -> a single wide DVE add.
- Neighbor features are fetched with indirect (gather) DMAs: one
  instruction gathers 128 rows (one per SBUF partition) from the feature
  table in DRAM at int32 row offsets.
- Feature tables are kept in a global slot-major layout [8*6272+1, 128]
  (zero row at the end for padding); after each layer the per-core
  h-shard is written back and exchanged with an AllGather collective
  ("halo exchange" degenerates to all-gather for a random graph).
- Weight matrices are replicated; the transform
  h = relu(agg@Wl.T + b + x@Wr.T) runs as two PE matmuls (feat-major)
  plus a fused bias+relu on the scalar engine. The three heads share one
  aggregation of h3 and run as a single concatenated [128, 28] matmul.
"""
import sys

sys.path.insert(0, "/opt/trn_rl_repo")

import numpy as np

N_CORES = 8
C = 128
WIN = 512  # transform window (moving free dim)


# ---------------------------------------------------------------- host prep
def prep_schedule(edge_index, n_nodes, n_cores=N_CORES):
    """Marshal edges into the per-core round/slot gather schedule."""
    src = np.asarray(edge_index[0], dtype=np.int64)
    dst = np.asarray(edge_index[1], dtype=np.int64)
    per_core = n_nodes // n_cores
    assert per_core * n_cores == n_nodes

    deg_all = np.bincount(dst, minlength=n_nodes).astype(np.int64)

    # per-core degree-sorted slot assignment
    S = ((per_core + 127) // 128) * 128  # slots per core (padded)
    node_of_slot = np.full((n_cores, S), -1, dtype=np.int64)
    slot_of_node = np.empty(n_nodes, dtype=np.int64)
    core_of_node = np.empty(n_nodes, dtype=np.int64)
    deg_slot = np.zeros((n_cores, S), dtype=np.int64)
    for c in range(n_cores):
        nodes = np.arange(c * per_core, (c + 1) * per_core)
        order = np.argsort(-deg_all[nodes], kind="stable")
        node_of_slot[c, :per_core] = nodes[order]
        slot_of_node[nodes[order]] = np.arange(per_core)
        core_of_node[nodes] = c
        deg_slot[c, :per_core] = deg_all[nodes[order]]

    NH = n_cores * S + 1  # global slot-major table rows (+1 zero row)
    ZROW = n_cores * S

    # CSR of incoming edges per node (by dst), then reindex by slot
    order_e = np.argsort(dst, kind="stable")
    src_sorted = src[order_e]
    starts = np.zeros(n_nodes + 1, dtype=np.int64)
    np.cumsum(np.bincount(dst, minlength=n_nodes), out=starts[1:])

    # global H-table row of a source node
    hrow_of_node = core_of_node * S + slot_of_node

    R = int(deg_slot.max())
    # words per round (max over cores so the program is uniform/SPMD)
    cnt = np.zeros((n_cores, R), dtype=np.int64)
    for c in range(n_cores):
        for r in range(R):
            cnt[c, r] = int((deg_slot[c] > r).sum())
    w_r = np.maximum(1, (cnt.max(axis=0) + 127) // 128).astype(np.int64)
    W_tot = int(w_r.sum())
    base_r = np.concatenate([[0], np.cumsum(w_r)])[:-1]

    # gather index tables: gidx[c][p, base_r + j] = H-row of the r-th
    # neighbor of the node in slot j*128+p (or ZROW pad)
    gidx = np.full((n_cores, 128, W_tot), ZROW, dtype=np.int32)
    for c in range(n_cores):
        for r in range(R):
            ns = int(cnt[c, r])
            if ns == 0:
                continue
            sl = node_of_slot[c, :ns]  # nodes in slots [0, ns) all have deg > r
            rows = hrow_of_node[src_sorted[starts[sl] + r]]
            nw = int(w_r[r])
            buf = np.full(nw * 128, ZROW, dtype=np.int32)
            buf[:ns] = rows
            gidx[c, :, base_r[r]:base_r[r] + nw] = buf.reshape(nw, 128).T

    degs = np.zeros((n_cores, 128, S // 128), dtype=np.float32)
    for c in range(n_cores):
        degs[c] = deg_slot[c].reshape(S // 128, 128).T.astype(np.float32)

    return dict(
        n_cores=n_cores, per_core=per_core, S=S, NH=NH, ZROW=ZROW,
        R=R, w_r=w_r, base_r=base_r, W_tot=W_tot,
        node_of_slot=node_of_slot, gidx=gidx, degs=degs,
        core_of_node=core_of_node, slot_of_node=slot_of_node,
    )


# ---------------------------------------------------------------- builder
def build_kernel(sch):
    import concourse.bacc as bacc
    import concourse.bass as bass
    import concourse.mybir as mybir
    import concourse.tile as tile

    n_cores, S, NH, W_tot = sch["n_cores"], sch["S"], sch["NH"], sch["W_tot"]
    NW = S // 128  # words per core
    w_r, base_r, R = sch["w_r"], sch["base_r"], sch["R"]
    f32 = mybir.dt.float32

    nc = bacc.Bacc("TRN2", target_bir_lowering=False, debug=False,
                   num_devices=n_cores)

    xt = nc.dram_tensor("xt", [NH, C], f32, kind="ExternalInput")
    xts = nc.dram_tensor("xts", [128, S], f32, kind="ExternalInput")
    gidx = nc.dram_tensor("gidx", [128, W_tot], mybir.dt.int32,
                          kind="ExternalInput")
    degs = nc.dram_tensor("degs", [128, NW], f32, kind="ExternalInput")
    wl = nc.dram_tensor("wl", [C, 3 * C], f32, kind="ExternalInput")
    wr = nc.dram_tensor("wr", [C, 3 * C], f32, kind="ExternalInput")
    bias = nc.dram_tensor("bias", [C, 3], f32, kind="ExternalInput")
    wh = nc.dram_tensor("wh", [C, 28], f32, kind="ExternalInput")
    whr = nc.dram_tensor("whr", [C, 28], f32, kind="ExternalInput")
    bh = nc.dram_tensor("bh", [C, 1], f32, kind="ExternalInput")
    ident_in = nc.dram_tensor("ident", [128, 128], f32, kind="ExternalInput")

    out_slots = nc.dram_tensor("out_slots", [S, 28], f32, kind="ExternalOutput")

    h_a = nc.dram_tensor("h_a", [NH, C], f32, addr_space="Shared")
    h_b = nc.dram_tensor("h_b", [NH, C], f32, addr_space="Shared")
    hshard = nc.dram_tensor("hshard", [S, C], f32)

    relu = mybir.ActivationFunctionType.Relu
    identf = mybir.ActivationFunctionType.Identity

    with tile.TileContext(nc) as tc:
        with tc.tile_pool(name="gp", bufs=2) as gp, \
             tc.tile_pool(name="work", bufs=1) as wk, \
             tc.tile_pool(name="ht", bufs=2) as htp, \
             tc.tile_pool(name="stage", bufs=4) as stp, \
             tc.tile_pool(name="ps1", bufs=4, space="PSUM") as ps1, \
             tc.tile_pool(name="ps2", bufs=2, space="PSUM") as ps2:

            # ---- persistent tiles
            idx_sb = wk.tile([128, W_tot], mybir.dt.int32)
            nc.sync.dma_start(out=idx_sb[:], in_=gidx[:])
            inv_sb = wk.tile([128, NW], f32)
            nc.sync.dma_start(out=inv_sb[:], in_=degs[:])
            nc.vector.tensor_scalar_max(inv_sb[:], inv_sb[:], 1.0)
            nc.vector.reciprocal(inv_sb[:], inv_sb[:])
            wl_sb = wk.tile([C, 3 * C], f32)
            nc.sync.dma_start(out=wl_sb[:], in_=wl[:])
            wr_sb = wk.tile([C, 3 * C], f32)
            nc.sync.dma_start(out=wr_sb[:], in_=wr[:])
            bias_sb = wk.tile([C, 3], f32)
            nc.sync.dma_start(out=bias_sb[:], in_=bias[:])
            wh_sb = wk.tile([C, 28], f32)
            nc.sync.dma_start(out=wh_sb[:], in_=wh[:])
            whr_sb = wk.tile([C, 28], f32)
            nc.sync.dma_start(out=whr_sb[:], in_=whr[:])
            bh_sb = wk.tile([C, 1], f32)
            nc.sync.dma_start(out=bh_sb[:], in_=bh[:])
            ident = wk.tile([128, 128], f32)
            nc.sync.dma_start(out=ident[:], in_=ident_in[:])
            acc = wk.tile([128, S], f32)
            aggT = wk.tile([128, S], f32)

            # zero rows of the shared h tables
            z0 = stp.tile([128, 128], f32, tag="z0")
            nc.vector.memset(z0[:], 0.0)
            nc.sync.dma_start(out=h_a[NH - 1:NH, :], in_=z0[0:1, :])
            nc.sync.dma_start(out=h_b[NH - 1:NH, :], in_=z0[0:1, :])

            # initial x^T (slot order) as the first "h^T prev"
            hT_prev = htp.tile([128, S], f32, tag="hT")
            nc.sync.dma_start(out=hT_prev[:], in_=xts[:])

            tbls = [xt, h_a, h_b, h_a]
            for p in range(4):
                tbl = tbls[p]
                # ---------- aggregation rounds
                nc.vector.memset(acc[:], 0.0)
                for r in range(R):
                    nw = int(w_r[r])
                    g = gp.tile([128, NW * 128], f32, tag="G")
                    for j in range(nw):
                        col = int(base_r[r]) + j
                        nc.gpsimd.indirect_dma_start(
                            out=g[:, j * 128:(j + 1) * 128],
                            out_offset=None,
                            in_=tbl[:],
                            in_offset=bass.IndirectOffsetOnAxis(
                                ap=idx_sb[:, col:col + 1], axis=0),
                        )
                    nc.vector.tensor_add(
                        out=acc[:, :nw * 128], in0=acc[:, :nw * 128],
                        in1=g[:, :nw * 128])
                # ---------- mean + transpose to feat-major
                for w in range(NW):
                    sl = slice(w * 128, (w + 1) * 128)
                    nc.vector.tensor_scalar_mul(
                        acc[:, sl], acc[:, sl], inv_sb[:, w:w + 1])
                    pt = ps1.tile([128, 128], f32, tag="pt")
                    nc.tensor.transpose(out=pt[:], in_=acc[:, sl],
                                        identity=ident[:])
                    nc.vector.tensor_copy(out=aggT[:, sl], in_=pt[:])
                # ---------- transform
                hT_cur = htp.tile([128, S], f32, tag="hT")
                if p < 3:
                    lw = wl_sb[:, p * C:(p + 1) * C]
                    rw = wr_sb[:, p * C:(p + 1) * C]
                    for w0 in range(0, S, WIN):
                        n = min(WIN, S - w0)
                        ps = ps2.tile([128, WIN], f32, tag="ps")
                        nc.tensor.matmul(ps[:, :n], lhsT=lw,
                                         rhs=aggT[:, w0:w0 + n],
                                         start=True, stop=False)
                        nc.tensor.matmul(ps[:, :n], lhsT=rw,
                                         rhs=hT_prev[:, w0:w0 + n],
                                         start=False, stop=True)
                        nc.scalar.activation(
                            out=hT_cur[:, w0:w0 + n], in_=ps[:, :n],
                            func=relu, bias=bias_sb[:, p:p + 1], scale=1.0)
                    # ---------- write back h shard (slot-major rows)
                    for w in range(NW):
                        sl = slice(w * 128, (w + 1) * 128)
                        pw = ps1.tile([128, 128], f32, tag="pt")
                        nc.tensor.transpose(out=pw[:], in_=hT_cur[:, sl],
                                            identity=ident[:])
                        st = stp.tile([128, 128], f32, tag="st")
                        nc.vector.tensor_copy(out=st[:], in_=pw[:])
                        nc.sync.dma_start(out=hshard[sl, :], in_=st[:])
                    h_next = tbls[p + 1]
                    nc.gpsimd.collective_compute(
                        "AllGather", mybir.AluOpType.bypass,
                        replica_groups=[list(range(n_cores))],
                        ins=[hshard[:]],
                        outs=[h_next[0:NH - 1, :]],
                    )
                else:
                    # heads: one concatenated [128, 28] transform, no relu
                    for w0 in range(0, S, WIN):
                        n = min(WIN, S - w0)
                        ps = ps2.tile([128, WIN], f32, tag="ps")
                        nc.tensor.matmul(ps[:28, :n], lhsT=wh_sb[:],
                                         rhs=aggT[:, w0:w0 + n],
                                         start=True, stop=False)
                        nc.tensor.matmul(ps[:28, :n], lhsT=whr_sb[:],
                                         rhs=hT_prev[:, w0:w0 + n],
                                         start=False, stop=True)
                        nc.scalar.activation(
                            out=hT_cur[:28, w0:w0 + n], in_=ps[:28, :n],
                            func=identf, bias=bh_sb[:28, 0:1], scale=1.0)
                    for w in range(NW):
                        sl = slice(w * 128, (w + 1) * 128)
                        po = ps1.tile([128, 128], f32, tag="pt")
                        nc.tensor.transpose(out=po[:, :28],
                                            in_=hT_cur[:28, sl],
                                            identity=ident[:28, :28])
                        st = stp.tile([128, 128], f32, tag="st")
                        nc.vector.tensor_copy(out=st[:, :28], in_=po[:, :28])
                        nc.sync.dma_start(out=out_slots[sl, :],
                                          in_=st[:, :28])
                hT_prev = hT_cur

    nc.compile()
    return nc


def empty_baseline():
    """Wall time of a trivial kernel execute (dispatch overhead)."""
    import concourse.bacc as bacc
    import concourse.mybir as mybir
    import concourse.tile as tile
    from runner import SpmdRunner
    nc = bacc.Bacc("TRN2", target_bir_lowering=False, debug=False,
                   num_devices=N_CORES)
    a = nc.dram_tensor("a", [128, 128], mybir.dt.float32, kind="ExternalInput")
    o = nc.dram_tensor("o", [128, 128], mybir.dt.float32, kind="ExternalOutput")
    with tile.TileContext(nc) as tc:
        with tc.tile_pool(name="p", bufs=1) as pool:
            t = pool.tile([128, 128], mybir.dt.float32)
            nc.sync.dma_start(out=t[:], in_=a[:])
            nc.sync.dma_start(out=o[:], in_=t[:])
    nc.compile()
    r = SpmdRunner(nc, N_CORES)
    t_min, _ = r.time_it(
        [{"a": np.zeros((128, 128), np.float32)}] * N_CORES,
        warmup=2, iters=7)
    return t_min


# ---------------------------------------------------------------- kernel
def kernel(**inputs):
    x = np.asarray(inputs["x"], dtype=np.float32)
    edge_index = np.asarray(inputs["edge_index"])
    n_nodes = x.shape[0]

    sch = prep_schedule(edge_index, n_nodes)
    n_cores, S, NH, per_core = (sch["n_cores"], sch["S"], sch["NH"],
                                sch["per_core"])

    # x in global slot-major layout (replicated to every core)
    xt_full = np.zeros((NH, C), dtype=np.float32)
    for c in range(n_cores):
        real = sch["node_of_slot"][c, :per_core]
        xt_full[c * S:c * S + per_core] = x[real]

    ident = np.eye(128, dtype=np.float32)
    wl_cat = np.concatenate(
        [np.ascontiguousarray(inputs[f"Wl_{k}"].T) for k in ("l1", "l2", "l3")],
        axis=1).astype(np.float32)
    wr_cat = np.concatenate(
        [np.ascontiguousarray(inputs[f"Wr_{k}"].T) for k in ("l1", "l2", "l3")],
        axis=1).astype(np.float32)
    bias_cat = np.stack(
        [inputs[f"b_{k}"] for k in ("l1", "l2", "l3")], axis=1
    ).astype(np.float32)
    wh_cat = np.concatenate(
        [np.ascontiguousarray(inputs[f"Wl_{k}"].T) for k in ("age", "sex", "eth")],
        axis=1).astype(np.float32)
    whr_cat = np.concatenate(
        [np.ascontiguousarray(inputs[f"Wr_{k}"].T) for k in ("age", "sex", "eth")],
        axis=1).astype(np.float32)
    bh_cat = np.concatenate(
        [inputs[f"b_{k}"] for k in ("age", "sex", "eth")]
    ).astype(np.float32)
    bh_pad = np.zeros((C, 1), np.float32)
    bh_pad[:28, 0] = bh_cat

    in_maps = []
    for c in range(n_cores):
        xts_c = np.zeros((128, S), dtype=np.float32)
        xts_c[:, :] = xt_full[c * S:(c + 1) * S].T
        in_maps.append({
            "xt": xt_full,
            "xts": xts_c,
            "gidx": sch["gidx"][c],
            "degs": sch["degs"][c],
            "wl": wl_cat, "wr": wr_cat, "bias": bias_cat,
            "wh": wh_cat, "whr": whr_cat, "bh": bh_pad,
            "ident": ident,
        })

    nc = build_kernel(sch)
    global _cached
    _cached = (nc, in_maps, sch)

    from concourse.bass_utils import run_bass_kernel_spmd
    res = run_bass_kernel_spmd(nc, in_maps, list(range(n_cores)))

    out_age = np.empty((n_nodes, 21), dtype=np.float32)
    out_sex = np.empty((n_nodes, 2), dtype=np.float32)
    out_eth = np.empty((n_nodes, 5), dtype=np.float32)
    for c in range(n_cores):
        o = res.results[c]["out_slots"]  # [S, 28]
        real = sch["node_of_slot"][c, :per_core]
        out_age[real] = o[:per_core, :21]
        out_sex[real] = o[:per_core, 21:23]
        out_eth[real] = o[:per_core, 23:28]
    return (out_age, out_sex, out_eth)
